# revision 30
# baseline (speedup 1.0000x reference)
"""Trainium2 Bass kernel for nn_MultiHeadAttentionQuantum.

Math: the per-(batch,token,head) quantum circuit (RX(x_i+theta_i) encode, CNOT
ring, <Z_i> readout) collapses analytically via Heisenberg/Clifford conjugation:
    <Z_0> = prod_{i=1..7} cos(x_i + theta_i)
    <Z_w> = prod_{i=0..w} cos(x_i + theta_i)   (w >= 1)
so the "quantum head" is cosine prefix-products. Downstream it is a plain
16-head self-attention (q=k=v, d_k=8, no max-subtraction needed since
|score| <= sqrt(8)) plus an output projection.

Device pipeline (per core = one batch element):
  - prefix products -> XQ [token, (tb,h,w)] f32; transposed -> xqT [e, token]
  - 16 steps over (qb, kb) 128-token block pairs:
      scores: 16 matmuls [128,128] per step (4 head-groups x 4 masked heads)
      exp: 4 instrs [128,512] split across ACT (exact) and DVE/Pool
           (Schraudolph int16-bitcast approx, ~+-3%) -> PH fp16 [k', (s,h,q')]
      PV: per head, matmul lhsT=PH block (K=128 keys), rhs=xq16 slab (N=8)
          accumulated over kb into psU [q', (h,w)]; denominator via rhs=ones
          (N=1) into psZ.
  - per qb: recip(z), stride-0-broadcast normalize, PE transpose, projection
    matmul (W.T stationary), bias add on evac, per-qb DMA out.

Sharding: data-parallel over batch, one batch element per NeuronCore (B=8,
n_cores=8). combine_heads weights replicated. No collectives.
"""

import math
import sys

sys.path.insert(0, "/opt/trn_rl_repo")

import numpy as np

import concourse.bass as bass  # noqa: F401  (import keeps bass registered)
import concourse.tile as tile
from concourse import bacc, mybir
from concourse import bass_utils

FP32 = mybir.dt.float32
FP16 = mybir.dt.float16
I16 = mybir.dt.int16
AF = mybir.ActivationFunctionType
MUL = mybir.AluOpType.mult
ADD = mybir.AluOpType.add

B, S, E, H, NW = 8, 512, 128, 16, 8   # batch, seq, embed, heads, wires(d_k)
TB = S // 128                         # token blocks = 4
ISQ = 1.0 / math.sqrt(NW)

# Schraudolph fp16 exp: bitcast(int16(round(s*C1 + C0))) ~= exp(s*ISQ), +-3%
C1 = 1024.0 * 1.4426950408889634 * ISQ
C0 = 15.0 * 1024.0 - 44.0

# Symmetry mirrors: exp(S) is symmetric since q=k, so for a mirrored pair the
# slab computed in the earlier-processed row is DMA-transposed (per-head,
# SBUF->SBUF) into its twin, whose scores+exp are skipped. Rows are processed
# in ROW_ORDER; row 3 first (computed fully, feeding three mirrors) keeps the
# exp engines evenly fed instead of starving in a mirror-heavy final row.
ROW_ORDER = [0, 3, 1, 2]
MIRROR_PAIRS = [(0, 1), (0, 2), (0, 3), (1, 3)]
_pos = {qb: i for i, qb in enumerate(ROW_ORDER)}
MIRROR_DST = {}
MIRROR_SRC = {}
for (a, b) in MIRROR_PAIRS:
    sr, dr = (a, b) if _pos[a] < _pos[b] else (b, a)
    src, dst = 4 * sr + dr, 4 * dr + sr   # slab ids (4*qb + kb)
    MIRROR_DST[dst] = src
    MIRROR_SRC[src] = dst

# exp engine assignment for non-mirrored (step, tile) units.
# 'A' = ACT exact exp, 'V' = DVE schraudolph int16-bitcast approx.
EXP_COUNTS = {"A": 25, "V": 23}


def _exp_sched():
    acc = {k: 0.0 for k in EXP_COUNTS}
    left = dict(EXP_COUNTS)
    total = sum(EXP_COUNTS.values())
    out = []
    for _ in range(total):
        for k in acc:
            acc[k] += left[k]
        pick = max(acc, key=lambda k: acc[k])
        acc[pick] -= total
        left[pick] -= 1
        out.append(pick)
    return out


SCHED = _exp_sched()

_CACHE = {}


def build(repeat: int = 1):
    if repeat in _CACHE:
        return _CACHE[repeat]

    nc = bacc.Bacc("TRN2", target_bir_lowering=False, debug=False, num_devices=8)

    xin_d = nc.dram_tensor("xin", [128, 512], FP32, kind="ExternalInput").ap()
    idn32_d = nc.dram_tensor("idn32", [128, 128], FP32, kind="ExternalInput").ap()
    idn16_d = nc.dram_tensor("idn16", [128, 128], FP16, kind="ExternalInput").ap()
    wtb_d = nc.dram_tensor("wtb", [128, 128], FP16, kind="ExternalInput").ap()
    msk_d = nc.dram_tensor("msk", [128, 4], FP32, kind="ExternalInput").ap()
    bvec_d = nc.dram_tensor("bvec", [128, 1], FP32, kind="ExternalInput").ap()
    yout_d = nc.dram_tensor("yout", [128, 512], FP32, kind="ExternalOutput").ap()

    with tile.TileContext(nc) as tc:
        with tc.tile_pool(name="consts", bufs=1) as cpool, \
             tc.tile_pool(name="sb", bufs=1) as spool, \
             tc.tile_pool(name="xop", bufs=2) as xopool, \
             tc.tile_pool(name="usp", bufs=2) as uspool, \
             tc.tile_pool(name="rzp", bufs=2) as rzpool, \
             tc.tile_pool(name="psS", bufs=6, space="PSUM") as psS, \
             tc.tile_pool(name="psFix", bufs=1, space="PSUM") as psF:

        # ---------------- input + consts (X first; two queues)
            X = spool.tile([128, 512], FP32, tag="X")
            idn32 = cpool.tile([128, 128], FP32, tag="idn32")
            msk = cpool.tile([128, 4], FP32, tag="msk")
            nc.sync.dma_start(X[:, 0:128], xin_d[:, 0:128])
            nc.sync.dma_start(X[:, 128:384], xin_d[:, 128:384])
            nc.sync.dma_start(msk[:], msk_d[:])
            idn16 = cpool.tile([128, 128], FP16, tag="idn16")
            wtb = cpool.tile([128, 128], FP16, tag="wtb")
            bvec = cpool.tile([128, 1], FP32, tag="bvec")
            nc.scalar.dma_start(idn32[:], idn32_d[:])
            nc.scalar.dma_start(idn16[:], idn16_d[:])
            nc.scalar.dma_start(wtb[:], wtb_d[:])
            nc.scalar.dma_start(bvec[:], bvec_d[:])
            nc.gpsimd.dma_start(X[:, 384:512], xin_d[:, 384:512])

            # ---------------- prefix products, per token block (pipelined)
            C = X
            XQ = spool.tile([128, 512], FP32, tag="XQ")
            Cr = C[:].rearrange("p (t h w) -> p t h w", t=TB, h=H, w=NW)
            Qr = XQ[:].rearrange("p (t h w) -> p t h w", t=TB, h=H, w=NW)
            scrT = spool.tile([128, 256], FP32, tag="scrT")
            Tr = scrT[:].rearrange("p (t h w) -> p t h w", t=TB, h=H, w=4)
            xq916 = spool.tile([128, 576], FP16, tag="xq916")
            xqT = spool.tile([128, 512], FP16, tag="xqT")
            Mv = [None] + [
                spool.tile([128, 512], FP16, tag=f"Mv{v}", name=f"Mv{v}")
                for v in range(1, 4)
            ]

            def emit_hs(t, eng):
                # Hillis-Steele prefix over wires 1..7 only, then fold c0 in
                # via stride-0 broadcast; never touches w0 (the Pool tree owns
                # it), so the DVE chain has no cross-engine hazard.
                q, c = Qr[:, t:t + 1], Cr[:, t:t + 1]
                eng.tensor_copy(q[:, :, :, 1:NW], c[:, :, :, 1:NW])
                for st in (1, 2, 4):
                    eng.tensor_mul(q[:, :, :, st + 1:NW], q[:, :, :, st + 1:NW],
                                   q[:, :, :, 1:NW - st])
                eng.tensor_mul(q[:, :, :, 1:NW], q[:, :, :, 1:NW],
                               c[:, :, :, 0:1].broadcast_to(
                                   [128, 1, H, NW - 1]))

            def emit_tree(t, eng):
                # wire 0 = suffix product of wires 1..7, 3-level tree
                q, c, tr = Qr[:, t:t + 1], Cr[:, t:t + 1], Tr[:, t:t + 1]
                eng.tensor_mul(tr[:, :, :, 0:1], c[:, :, :, 3:4], c[:, :, :, 4:5])
                eng.tensor_mul(tr[:, :, :, 1:2], c[:, :, :, 5:6], c[:, :, :, 6:7])
                eng.tensor_mul(tr[:, :, :, 2:3], c[:, :, :, 1:2], c[:, :, :, 2:3])
                eng.tensor_mul(tr[:, :, :, 3:4], tr[:, :, :, 0:1], tr[:, :, :, 1:2])
                eng.tensor_mul(tr[:, :, :, 3:4], tr[:, :, :, 3:4], c[:, :, :, 7:8])
                eng.tensor_mul(q[:, :, :, 0:1], tr[:, :, :, 2:3], tr[:, :, :, 3:4])

            def emit_block(t):
                # transpose block t via spare psMix cols, evac, per-block masks
                pT = psMix[:, 352:480]
                nc.tensor.transpose(pT, XQ[:, 128 * t:128 * (t + 1)], idn32[:])
                nc.vector.tensor_copy(xqT[:, 128 * t:128 * (t + 1)], pT)
                for v in range(1, 4):
                    nc.vector.tensor_scalar_mul(
                        Mv[v][:, 128 * t:128 * (t + 1)],
                        xqT[:, 128 * t:128 * (t + 1)], msk[:, v:v + 1])

            # ---------------- PH: exp'd score slabs [k', (step, h, q')] fp16
            PH = spool.tile([128, 16 * 2048], FP16, tag="PH")

            # psum fixed tiles: one mixed bank (psU/psZ ping-pong + psT), psOT
            psMix = psF.tile([128, 512], FP32, tag="psMix", name="psMix")
            psOT = psF.tile([128, 512], FP32, tag="psOT", name="psOT")

            def psU_r(qb):
                return psMix[:, 144 * (qb % 2):144 * (qb % 2) + 144]

            psT16 = psMix[:, 288:352].bitcast(FP16)   # [128, 128] fp16

            yo = spool.tile([128, 512], FP32, tag="yo")

            def emit_scores(s, tiles):
                kb, qb = s % 4, s // 4
                for g in range(4):
                    pt = psS.tile([128, 512], FP32, tag="ps_s",
                                  name=f"sc{s}g{g}")
                    tiles.append(pt)
                    for v in range(4):
                        if v == 0:
                            lhsT = xqT[32 * g:32 * g + 8, 128 * kb:128 * (kb + 1)]
                            rhs = xqT[32 * g:32 * g + 8, 128 * qb:128 * (qb + 1)]
                        else:
                            lhsT = Mv[v][32 * g:32 * (g + 1), 128 * kb:128 * (kb + 1)]
                            rhs = xqT[32 * g:32 * (g + 1), 128 * qb:128 * (qb + 1)]
                        nc.tensor.matmul(
                            pt[:, 128 * v:128 * (v + 1)], lhsT, rhs,
                            start=True, stop=True,
                            tile_position=(32 * g, 0), skip_group_check=True,
                        )

            unit_ctr = [0]

            def emit_exp(s, tiles):
                for g in range(4):
                    eng = SCHED[unit_ctr[0]]
                    unit_ctr[0] += 1
                    dst = PH[:, 2048 * s + 512 * g: 2048 * s + 512 * (g + 1)]
                    src = tiles[g]
                    if eng == "A":
                        nc.scalar.activation(dst, src[:], AF.Exp, scale=ISQ)
                    else:
                        nc.vector.tensor_scalar(dst.bitcast(I16), src[:],
                                                C1, C0, MUL, ADD)

            def emit_mirror(s_src):
                s_dst = MIRROR_SRC[s_src]
                for half in range(2):
                    src = PH[:, 2048 * s_src + 1024 * half:
                             2048 * s_src + 1024 * (half + 1)]
                    dst = PH[:, 2048 * s_dst + 1024 * half:
                             2048 * s_dst + 1024 * (half + 1)]
                    nc.sync.dma_start_transpose(
                        dst.rearrange("p (h q) -> p h q", h=8), src)

            def emit_pv_burst(qb):
                # h-outer / kb-inner: exactly one psum accumulation group is
                # open at a time (interleaved open groups drop partials).
                pu = psU_r(qb)
                for h in range(H):
                    for kb in range(4):
                        s = 4 * qb + kb
                        lhsT = PH[:, 2048 * s + 128 * h: 2048 * s + 128 * (h + 1)]
                        nc.tensor.matmul(
                            pu[:, 9 * h: 9 * (h + 1)],
                            lhsT,
                            xq916[:, 144 * kb + 9 * h: 144 * kb + 9 * (h + 1)],
                            start=(kb == 0), stop=(kb == 3), skip_group_check=True,
                        )

            def emit_chain(qb):
                rz = rzpool.tile([128, 16], FP32, tag="rz", name=f"rz{qb}")
                pu9 = psU_r(qb).rearrange("p (h w) -> p h w", w=NW + 1)
                nc.vector.reciprocal_approx_fast(
                    out=rz[:].unsqueeze(2), in_=pu9[:, :, NW:NW + 1])
                uS = uspool.tile([128, 128], FP16, tag="uS", name=f"uS{qb}")
                uv = pu9[:, :, 0:NW]
                rb = rz[:].unsqueeze(2).broadcast_to([128, H, NW])
                nc.vector.tensor_mul(
                    uS[:].rearrange("p (h w) -> p h w", h=H), uv, rb)
                tslot = psT16[:, 0:128]
                nc.tensor.transpose(tslot, uS[:], idn16[:])
                xo = xopool.tile([128, 128], FP16, tag="xo", name=f"xo{qb}")
                nc.scalar.activation(xo[:], tslot, AF.Copy)
                nc.tensor.matmul(
                    psOT[:, 128 * qb:128 * (qb + 1)], wtb[:], xo[:],
                    start=True, stop=True, skip_group_check=True,
                )
                nc.scalar.activation(
                    yo[:, 128 * qb:128 * (qb + 1)],
                    psOT[:, 128 * qb:128 * (qb + 1)],
                    AF.Identity, bias=bvec[:, 0:1])
                nc.sync.dma_start(
                    yout_d[:, 128 * qb:128 * (qb + 1)],
                    yo[:, 128 * qb:128 * (qb + 1)])

            # ---------------- prologue: blocks 0..1 tight, 2..3 behind step 0
            for t in (0, 1):
                emit_hs(t, nc.vector)
                emit_tree(t, nc.gpsimd)
                emit_block(t)

            for idx in range(16):
                qb, kb = ROW_ORDER[idx // 4], idx % 4
                s = 4 * qb + kb
                if idx == 1:
                    for t in (2, 3):
                        emit_hs(t, nc.vector)
                        emit_tree(t, nc.gpsimd)
                        emit_block(t)
                    x9 = xq916[:].rearrange("p (t h w) -> p t h w", t=TB, w=NW + 1)
                    nc.gpsimd.tensor_copy(
                        x9[:, :, :, 0:NW],
                        XQ[:].rearrange("p (t h w) -> p t h w", t=TB, w=NW))
                    nc.gpsimd.memset(x9[:, :, :, NW:NW + 1], 1.0)
                tiles = []
                if s not in MIRROR_DST:
                    emit_scores(s, tiles)
                if idx % 4 == 0 and idx > 0:
                    emit_pv_burst(ROW_ORDER[idx // 4 - 1])
                    emit_chain(ROW_ORDER[idx // 4 - 1])
                if s not in MIRROR_DST:
                    emit_exp(s, tiles)
                    if s in MIRROR_SRC:
                        emit_mirror(s)
            emit_pv_burst(ROW_ORDER[3])
            emit_chain(ROW_ORDER[3])

    nc.compile()
    _CACHE[repeat] = nc
    return nc


def _consts(W: np.ndarray, b: np.ndarray):
    return {
        "idn32": np.eye(128, dtype=np.float32),
        "idn16": np.eye(128, dtype=np.float16),
        "wtb": np.ascontiguousarray(W.T).astype(np.float16),
        "msk": np.eye(4, dtype=np.float32)[(np.arange(128) % 32) // 8].astype(np.float32),
        "bvec": b.reshape(128, 1).astype(np.float32),
    }


def _prep_x(x: np.ndarray, theta: np.ndarray) -> list[np.ndarray]:
    """Per-core xin: RX-encoding cosines cos(x + theta), laid out as
    [token_within_block, (block, embed)]."""
    theta_full = np.tile(theta.astype(np.float64), E // NW)
    a = np.cos(x.astype(np.float64) + theta_full).astype(np.float32)
    return [
        np.ascontiguousarray(
            a[bb].reshape(TB, 128, E).transpose(1, 0, 2).reshape(128, TB * E)
        )
        for bb in range(B)
    ]


def kernel(x: np.ndarray, theta: np.ndarray, W: np.ndarray, b: np.ndarray) -> np.ndarray:
    x = np.asarray(x, dtype=np.float32)
    theta = np.asarray(theta, dtype=np.float32)
    W = np.asarray(W, dtype=np.float32)
    b = np.asarray(b, dtype=np.float32)

    nc = build(repeat=1)
    consts = _consts(W, b)
    xins = _prep_x(x, theta)
    in_maps = [{**consts, "xin": xins[c]} for c in range(B)]
    res = bass_utils.run_bass_kernel_spmd(nc, in_maps, core_ids=list(range(8)))

    y = np.empty((B, S, E), dtype=np.float32)
    for c in range(B):
        y[c] = res.results[c]["yout"].T  # [e', q] -> [q, e']
    return y
